# revision 16
# baseline (speedup 1.0000x reference)
"""Trainium2 Bass kernel: per-species expert linear + structure segment-sum.

Math: out[g] = sum_{atoms i in structure g} (x[i] @ W[species_i] + b[species_i])
Linear => aggregate first, matmul after:
  A[g, s, :] = sum_{i: struct_i=g, species_i=s} x[i]            (256-dim)
  out[g]     = sum_s A[g, s, :] @ W_s  +  count[g, s] * b[s]
The bias term count[g,s]*b[s] is added on HOST (counts are index-only).

Stage 1 (device): segment-sum of x rows by combined seg = 4*struct + species
via one-hot matmuls accumulating in PSUM. Atoms are pre-sorted by struct, so
a 128-atom tile touches a narrow seg band; PSUM holds a sliding 128-seg
window. Schedule is host-computed from the indices, unioned across the 8
cores so the SPMD graph is identical on every core.

One-hot masks depend only on the indices (not on x), so ALL masks are built
up-front on the DVE while x streams from HBM: one batched tensor_tensor
is_equal per K=8 blocks (iota-vs-preshifted-seg, k-interleaved layout so
both TT operands keep stride-1 last dims = DVE fast mode, ~90ns/block).
Stage-1 matmuls then use stride-K stationary slices (measured 113ns/MM,
faster than contiguous). x streams in bf16 (one-hot exact in bf16; rel err
~3e-3), PSUM accumulates f32.

Stage 2 (device): PE-transpose window accumulators to feature-major (tk),
then per QUAD of windows (128 structs) contract against packed expert
weights: 8 matmuls (2 f-chunks x 4 species) with stride-4 stationary.
Emitted as soon as a quad's windows flush so the tail stays short.

Sharding: 25000 contiguous atoms per core (structs stay contiguous because
structural_indices are sorted); host overlap-adds the 8 partial per-struct
outputs and adds the exact bias term. x is packed partition-major on host so
every DMA descriptor is one contiguous run per SBUF partition.
"""

import numpy as np

P = 128
N_ATOMS = 200_000
D_IN = 256
D_OUT = 256
N_SPECIES = 4
N_STRUCT = 2_000
N_CORES = 8
SH = N_ATOMS // N_CORES            # atoms per core
TPC = (SH + P - 1) // P            # tiles per core
SH_PAD = TPC * P                   # padded atoms per core
RAMP = [3, 4, 7, 14]               # small leading chunks: compute starts early
CH = 28                            # x tiles per big middle chunk
TAIL_CHUNK = 2                     # smaller final chunks shorten the tail
TAIL_TILES = 4                     # how many trailing tiles use TAIL_CHUNK
CHUNK_BUFS = 5
AT_BUFS = 2
TP_BUFS = 2
PO_BUFS = 2
K = 8                              # blocks per batched mask build


def _schedule(seg_local_real):
    """seg_local_real: list of per-core int arrays [SH] of local seg ids.
    Returns NW (num 128-seg windows, multiple of 4), ordered block list
    [(t, w)], first/last tile per window, and max concurrently-live windows.
    All unioned across cores so the SPMD graph is identical."""
    max_seg = max(int(s.max()) for s in seg_local_real)
    n_win = (max_seg + 1 + P - 1) // P
    NW = ((n_win + 3) // 4) * 4

    wlo = np.full(TPC, 1 << 30, np.int64)
    whi = np.full(TPC, -1, np.int64)
    for s in seg_local_real:
        for t in range(TPC):
            a0, a1 = t * P, min((t + 1) * P, SH)
            tl = s[a0:a1]
            wlo[t] = min(wlo[t], int(tl.min()) // P)
            whi[t] = max(whi[t], int(tl.max()) // P)

    blocks = []
    first_tile = {}
    last_tile = {}
    for t in range(TPC):
        for w in range(int(wlo[t]), int(whi[t]) + 1):
            blocks.append((t, w))
            if w not in first_tile:
                first_tile[w] = t
            last_tile[w] = t
    max_alive = max(int(whi[t] - wlo[t]) + 1 for t in range(TPC))
    return NW, blocks, first_tile, last_tile, max_alive


def _build(NW, blocks, first_tile, last_tile, win_bufs, reps=1):
    import contextlib

    import concourse.bacc as bacc
    import concourse.mybir as mybir
    import concourse.tile as tile
    from concourse.bass import AP

    f32 = mybir.dt.float32
    bf16 = mybir.dt.bfloat16
    NB = (len(blocks) + K - 1) // K       # mask build groups
    NBK = NB * K                          # padded block count
    NWG = NW // 4                         # stage-2 quads

    nc = bacc.Bacc(None, target_bir_lowering=False)
    xp_d = nc.declare_dram_parameter("xp", [P, TPC * D_IN], bf16, isOutput=False)
    # pre: preshifted seg per block [P, NBK] int16 (iota built on device,
    # is_equal compares int16 directly -- no cast on the critical path)
    pc_d = nc.declare_dram_parameter("pc", [P, NBK], mybir.dt.int16, isOutput=False)
    wk_d = nc.declare_dram_parameter("wk", [P, 8 * D_OUT], bf16, isOutput=False)
    id_d = nc.declare_dram_parameter("ident", [P, P], bf16, isOutput=False)
    out_d = nc.declare_dram_parameter("out", [NW * 32, D_OUT], bf16, isOutput=True)

    with tile.TileContext(nc) as tc:
        with (
            tc.tile_pool(name="const", bufs=1) as constp,
            tc.tile_pool(name="chunk", bufs=CHUNK_BUFS) as chunkp,
            tc.tile_pool(name="masks", bufs=1) as maskp,
            tc.tile_pool(name="atmp", bufs=AT_BUFS) as atp,
            tc.tile_pool(name="tks", bufs=1) as tkp,
            tc.tile_pool(name="win", bufs=win_bufs, space="PSUM") as winp,
            tc.tile_pool(name="tp", bufs=TP_BUFS, space="PSUM") as tpp,
            tc.tile_pool(name="po", bufs=PO_BUFS, space="PSUM") as pop,
        ):
            # tiny consts needed by the mask builds go first, on the Scalar
            # hw-DGE queue so x chunks own the Sync queue in strict order
            pc_sb = constp.tile([P, NBK], mybir.dt.int16)
            nc.scalar.dma_start(pc_sb[:], pc_d[:])
            # iota-repeated-K operand for the mask builds, on the idle GpSimd
            # queue so it is ready before the pre DMA lands
            iotar = constp.tile([P, P * K], mybir.dt.int16)
            nc.gpsimd.iota(iotar[:], pattern=[[1, P], [0, K]], channel_multiplier=0)
            # larger consts only needed at first flush / stage 2
            ident_sb = constp.tile([P, P], bf16)
            wk_sb = constp.tile([P, 8 * D_OUT], bf16)

            masks = maskp.tile([P, NBK * P], bf16, tag="masks")
            tk0 = tkp.tile([P, NW * P], bf16, tag="tk0")
            tk1 = tkp.tile([P, NW * P], bf16, tag="tk1")

            loop_cm = (
                tc.For_i(
                    0,
                    reps,
                    1,
                    hint_engines=(
                        mybir.EngineType.PE,
                        mybir.EngineType.DVE,
                        mybir.EngineType.Activation,
                        mybir.EngineType.SP,
                    ),
                )
                if reps > 1
                else contextlib.nullcontext()
            )
            first_body = [True, True]
            with loop_cm:
                _emit_body(
                    nc, tc, mybir, AP, f32, bf16, NW, NB, NWG, blocks,
                    first_tile, last_tile, chunkp, atp, winp, tpp, pop,
                    pc_sb, iotar, ident_sb, wk_sb, masks, tk0, tk1,
                    xp_d, out_d, id_d, wk_d, first_body,
                )

    nc.compile()
    return nc


def _emit_body(
    nc, tc, mybir, AP, f32, bf16, NW, NB, NWG, blocks, first_tile, last_tile,
    chunkp, atp, winp, tpp, pop, pc_sb, iotar, ident_sb, wk_sb, masks, tk0, tk1,
    xp_d, out_d, id_d, wk_d, first_body,
):
    NBK = NB * K
    pc_t = pc_sb[:].tensor
    iot_t = iotar[:].tensor
    masks_t = masks[:].tensor

    # all mask builds up-front: depend only on pc, run while x streams.
    # out[p, c*K + k] = (pre[p, gK+k] == c)  -- k-interleaved, both TT
    # operands keep stride-1 last dims (DVE fast mode).
    for g in range(NB):
        in0 = AP(iot_t, 0, [[P * K, P], [1, P * K]])
        in1 = AP(pc_t, g * K, [[NBK, P], [0, P], [1, K]])
        nc.vector.tensor_tensor(
            out=masks[:, g * K * P : (g + 1) * K * P],
            in0=in0,
            in1=in1,
            op=mybir.AluOpType.is_equal,
        )

    quad_flushed = {g: 0 for g in range(NWG)}
    quads_done = set()
    quad_emitted = {}
    po_tiles = {}
    real_windows = set(first_tile)

    last_real_g = max(first_tile) // 4

    def emit_stage2_quad(g, w_hi=None):
        # windows [4g, 4g+4) -> 128 structs -> output rows [128g, 128g+128)
        # w_hi: restrict to windows [4g, w_hi] (partial emit for the final
        # quad so only the last window's matmuls sit in the serial tail)
        po = po_tiles.get(g)
        if po is None:
            po = po_tiles[g] = pop.tile([P, D_OUT], f32, tag="po", name=f"po{g}")
        w0 = quad_emitted.get(g, 4 * g)
        w1 = 4 * g + 4 if w_hi is None else w_hi + 1
        quad_emitted[g] = w1
        for kc, tkbuf in ((0, tk0), (1, tk1)):
            for s in range(N_SPECIES):
                nc.tensor.matmul(
                    po[32 * (w0 - 4 * g) : 32 * (w1 - 4 * g), :],
                    lhsT=tkbuf[:, w0 * P + s : w1 * P : 4],
                    rhs=wk_sb[:, (s * 2 + kc) * D_OUT : (s * 2 + kc + 1) * D_OUT],
                    start=(kc == 0 and s == 0),
                    stop=(kc == 1 and s == N_SPECIES - 1),
                    tile_position=(0, 32 * (w0 - 4 * g)) if w0 != 4 * g or w1 != 4 * g + 4 else None,
                )
        if w1 < 4 * g + 4:
            return
        quads_done.add(g)
        ob = atp.tile([P, D_OUT], bf16, tag="ob", name=f"ob{g}")
        nc.scalar.copy(ob[:], po[:])
        nc.sync.dma_start(out_d[g * P : (g + 1) * P, :], ob[:])

    # chunk plan: small ramp chunks so compute starts early, big middle
    # chunks to amortize the ~600ns DMA trigger cost, small tail chunks so
    # the final tiles (the serial tail) start as soon as possible
    plan = list(RAMP)
    body_end = TPC - TAIL_TILES
    while sum(plan) + CH <= body_end:
        plan.append(CH)
    if sum(plan) < body_end:
        plan.append(body_end - sum(plan))
    while sum(plan) + TAIL_CHUNK <= TPC:
        plan.append(TAIL_CHUNK)
    if sum(plan) < TPC:
        plan.append(TPC - sum(plan))
    starts = list(np.cumsum([0] + plan[:-1]))
    sizes = dict(zip(starts, plan))


    # block index lookup: blocks are ordered by (t, w)
    block_idx = {b: j for j, b in enumerate(blocks)}
    tile_windows = {}
    for t, w in blocks:
        tile_windows.setdefault(t, []).append(w)

    psw = {}
    chunk = None
    coff = 0
    for t in range(TPC):
        if t in sizes:
            csz = sizes[t]
            chunk = chunkp.tile([P, CH * D_IN], bf16, tag="chunk", name=f"ch{t}")
            nc.sync.dma_start(
                chunk[:, : csz * D_IN], xp_d[:, t * D_IN : (t + csz) * D_IN]
            )
            coff = t
        if t == 0 and first_body[0]:
            # ident on the scalar queue (tiny); tk zero-fill on idle GpSimd
            # so the DVE starts mask builds immediately; wk (0.5 MB) is
            # deferred to the first flush to keep it off the early stream
            first_body[0] = False
            nc.scalar.dma_start(ident_sb[:], id_d[:])
            nc.gpsimd.memset(tk0[:], 0.0)
            nc.gpsimd.memset(tk1[:], 0.0)
        xt = chunk[:, (t - coff) * D_IN : (t - coff + 1) * D_IN]
        for w in tile_windows[t]:
            if w not in psw:
                psw[w] = winp.tile([P, D_IN], f32, tag="win", name=f"win{w}")
            j = block_idx[(t, w)]
            g, k = j // K, j % K
            lhsT = AP(masks_t, g * K * P + k, [[NBK * P, P], [K, P]])
            nc.tensor.matmul(
                psw[w][:],
                lhsT=lhsT,
                rhs=xt,
                start=(t == first_tile[w]),
                stop=(t == last_tile[w]),
            )
        # flush finished windows: transpose into feature-major tk buffers
        for w in sorted(psw):
            if t != last_tile[w]:
                continue
            at = atp.tile([P, D_IN], bf16, tag="at")
            nc.scalar.copy(at[:], psw[w][:])
            if first_body[1]:
                first_body[1] = False
                nc.scalar.dma_start(wk_sb[:], wk_d[:])
            for kc, tkbuf in ((0, tk0), (1, tk1)):
                # hardware XBAR transpose (SBUF->SBUF DMA) keeps the PE free
                nc.scalar.dma_start_transpose(
                    tkbuf[:, w * P : (w + 1) * P], at[:, kc * P : (kc + 1) * P]
                )
            del psw[w]
            g = w // 4
            quad_flushed[g] += 1
            n_real = sum(1 for ww in range(4 * g, 4 * g + 4) if ww in real_windows)
            if quad_flushed[g] == n_real:
                emit_stage2_quad(g)
            elif g == last_real_g and quad_flushed[g] == n_real - 1:
                # final quad: pre-emit all but its last window mid-stream
                emit_stage2_quad(g, w_hi=4 * g + n_real - 2)

    # quads with no real windows (NW padding): zeros via tk memset
    for g in range(NWG):
        if g not in quads_done:
            emit_stage2_quad(g)


def _prep(x, W, b, central_species, structural_indices):
    """Host-side prep: schedule from indices + packed per-core in_maps."""
    import ml_dtypes

    x = np.asarray(x, dtype=np.float32)
    Wf = np.asarray(W, dtype=np.float32)
    bf = np.asarray(b, dtype=np.float32)
    cs = np.asarray(central_species).astype(np.int64)
    si = np.asarray(structural_indices).astype(np.int64)

    if not np.all(np.diff(si) >= 0):
        order = np.argsort(si, kind="stable")
        si = si[order]
        cs = cs[order]
        x = x[order]

    seg = 4 * si + cs
    g0 = [int(si[c * SH]) for c in range(N_CORES)]
    seg_local_real = [
        (seg[c * SH : (c + 1) * SH] - 4 * g0[c]).astype(np.int64)
        for c in range(N_CORES)
    ]
    NW, blocks, first_tile, last_tile, max_alive = _schedule(seg_local_real)
    win_bufs = 4
    NB = (len(blocks) + K - 1) // K
    NBK = NB * K

    bf16 = ml_dtypes.bfloat16
    ident = np.eye(P, dtype=bf16)
    wk = np.zeros((P, 8, D_OUT), bf16)
    for s in range(N_SPECIES):
        for kc in range(2):
            wk[:, s * 2 + kc, :] = Wf[s, kc * P : (kc + 1) * P, :].astype(bf16)
    wk = np.ascontiguousarray(wk.reshape(P, 8 * D_OUT))

    in_maps = []
    for c in range(N_CORES):
        xp = np.zeros((SH_PAD, D_IN), np.float32)
        xp[:SH] = x[c * SH : (c + 1) * SH]
        # partition-major: SBUF partition p holds atoms {t*128+p} contiguously
        xp = np.ascontiguousarray(
            xp.reshape(TPC, P, D_IN).transpose(1, 0, 2).reshape(P, TPC * D_IN)
        ).astype(bf16)
        segt = np.full((TPC, P), -(1 << 20), np.float32)
        segt.reshape(-1)[:SH] = seg_local_real[c].astype(np.float32)
        pre = np.full((P, NBK), -1.0, np.float32)
        for j, (t, w) in enumerate(blocks):
            v = segt[t] - P * w
            pre[:, j] = np.where((v >= 0) & (v < P), v, -1.0)
        in_maps.append(
            {"xp": xp, "pc": pre.astype(np.int16), "wk": wk, "ident": ident}
        )

    # exact bias term on host: out[g] += sum_s count[g, s] * b[s]
    counts = np.bincount(seg, minlength=4 * N_STRUCT).reshape(N_STRUCT, 4)
    host_bias = counts.astype(np.float32) @ bf

    return {
        "build_args": (NW, blocks, first_tile, last_tile, win_bufs),
        "in_maps": in_maps,
        "g0": g0,
        "NW": NW,
        "host_bias": host_bias,
    }


def kernel(x, W, b, central_species, structural_indices):
    from concourse.bass_utils import run_bass_kernel_spmd

    prep = _prep(x, W, b, central_species, structural_indices)
    nc = _build(*prep["build_args"])
    res = run_bass_kernel_spmd(
        nc, prep["in_maps"], core_ids=list(range(N_CORES))
    )

    g0, NW = prep["g0"], prep["NW"]
    full = np.zeros((N_STRUCT + NW * 32, D_OUT), np.float32)
    for c in range(N_CORES):
        full[g0[c] : g0[c] + NW * 32] += np.asarray(
            res.results[c]["out"], dtype=np.float32
        )
    full[:N_STRUCT] += prep["host_bias"]
    return np.ascontiguousarray(full[:N_STRUCT])


# revision 17
# speedup vs baseline: 1.7431x; 1.7431x over previous
"""Trainium2 Bass kernel: per-species expert linear + structure segment-sum.

Math: out[g] = sum_{atoms i in structure g} (x[i] @ W[species_i] + b[species_i])
Linear => aggregate first, matmul after:
  A[g, s, :] = sum_{i: struct_i=g, species_i=s} x[i]            (256-dim)
  out[g]     = sum_s A[g, s, :] @ W_s  +  count[g, s] * b[s]
The bias term count[g,s]*b[s] is added on HOST (counts are index-only).

Stage 1 (device): segment-sum of x rows by combined seg = 4*struct + species
via one-hot matmuls accumulating in PSUM. Atoms are pre-sorted by struct, so
a 128-atom tile touches a narrow seg band; PSUM holds a sliding 128-seg
window. Schedule is host-computed from the indices, unioned across the 8
cores so the SPMD graph is identical on every core.

One-hot masks depend only on the indices (not on x), so ALL masks are built
up-front on the DVE while x streams from HBM: one batched tensor_tensor
is_equal per K=8 blocks (iota-vs-preshifted-seg, k-interleaved layout so
both TT operands keep stride-1 last dims = DVE fast mode, ~90ns/block).
Stage-1 matmuls then use stride-K stationary slices (measured 113ns/MM,
faster than contiguous). x streams in bf16 (one-hot exact in bf16; rel err
~3e-3), PSUM accumulates f32.

Stage 2 (device): PE-transpose window accumulators to feature-major (tk),
then per QUAD of windows (128 structs) contract against packed expert
weights: 8 matmuls (2 f-chunks x 4 species) with stride-4 stationary.
Emitted as soon as a quad's windows flush so the tail stays short.

Sharding: 25000 contiguous atoms per core (structs stay contiguous because
structural_indices are sorted); host overlap-adds the 8 partial per-struct
outputs and adds the exact bias term. x is packed partition-major on host so
every DMA descriptor is one contiguous run per SBUF partition.
"""

import numpy as np

P = 128
N_ATOMS = 200_000
D_IN = 256
D_OUT = 256
N_SPECIES = 4
N_STRUCT = 2_000
N_CORES = 8
SH = N_ATOMS // N_CORES            # atoms per core
TPC = (SH + P - 1) // P            # tiles per core
SH_PAD = TPC * P                   # padded atoms per core
RAMP = [3, 4, 7, 14]               # small leading chunks: compute starts early
CH = 28                            # x tiles per big middle chunk
TAIL_CHUNK = 2                     # smaller final chunks shorten the tail
TAIL_TILES = 4                     # how many trailing tiles use TAIL_CHUNK
CHUNK_BUFS = 5
AT_BUFS = 2
TP_BUFS = 2
PO_BUFS = 2
K = 8                              # blocks per batched mask build


def _schedule(seg_local_real):
    """seg_local_real: list of per-core int arrays [SH] of local seg ids.
    Returns NW (num 128-seg windows, multiple of 4), ordered block list
    [(t, w)], first/last tile per window, and max concurrently-live windows.
    All unioned across cores so the SPMD graph is identical."""
    max_seg = max(int(s.max()) for s in seg_local_real)
    n_win = (max_seg + 1 + P - 1) // P
    NW = ((n_win + 3) // 4) * 4

    wlo = np.full(TPC, 1 << 30, np.int64)
    whi = np.full(TPC, -1, np.int64)
    for s in seg_local_real:
        for t in range(TPC):
            a0, a1 = t * P, min((t + 1) * P, SH)
            tl = s[a0:a1]
            wlo[t] = min(wlo[t], int(tl.min()) // P)
            whi[t] = max(whi[t], int(tl.max()) // P)

    blocks = []
    first_tile = {}
    last_tile = {}
    for t in range(TPC):
        for w in range(int(wlo[t]), int(whi[t]) + 1):
            blocks.append((t, w))
            if w not in first_tile:
                first_tile[w] = t
            last_tile[w] = t
    max_alive = max(int(whi[t] - wlo[t]) + 1 for t in range(TPC))
    return NW, blocks, first_tile, last_tile, max_alive


def _build(NW, blocks, first_tile, last_tile, win_bufs, reps=1):
    import contextlib

    import concourse.bacc as bacc
    import concourse.mybir as mybir
    import concourse.tile as tile
    from concourse.bass import AP

    f32 = mybir.dt.float32
    bf16 = mybir.dt.bfloat16
    NB = (len(blocks) + K - 1) // K       # mask build groups
    NBK = NB * K                          # padded block count
    NWG = NW // 4                         # stage-2 quads

    nc = bacc.Bacc(None, target_bir_lowering=False)
    xp_d = nc.declare_dram_parameter("xp", [P, TPC * D_IN], bf16, isOutput=False)
    # pre: preshifted seg per block [P, NBK] int16 (iota built on device,
    # is_equal compares int16 directly -- no cast on the critical path)
    pc_d = nc.declare_dram_parameter("pc", [P, NBK], mybir.dt.int16, isOutput=False)
    wk_d = nc.declare_dram_parameter("wk", [P, 8 * D_OUT], bf16, isOutput=False)
    id_d = nc.declare_dram_parameter("ident", [P, P], bf16, isOutput=False)
    out_d = nc.declare_dram_parameter("out", [NW * 32, D_OUT], bf16, isOutput=True)

    with tile.TileContext(nc) as tc:
        with (
            tc.tile_pool(name="const", bufs=1) as constp,
            tc.tile_pool(name="chunk", bufs=CHUNK_BUFS) as chunkp,
            tc.tile_pool(name="masks", bufs=1) as maskp,
            tc.tile_pool(name="atmp", bufs=AT_BUFS) as atp,
            tc.tile_pool(name="tks", bufs=1) as tkp,
            tc.tile_pool(name="win", bufs=win_bufs, space="PSUM") as winp,
            tc.tile_pool(name="tp", bufs=TP_BUFS, space="PSUM") as tpp,
            tc.tile_pool(name="po", bufs=PO_BUFS, space="PSUM") as pop,
        ):
            # tiny consts needed by the mask builds go first, on the Scalar
            # hw-DGE queue so x chunks own the Sync queue in strict order
            pc_sb = constp.tile([P, NBK], mybir.dt.int16)
            nc.scalar.dma_start(pc_sb[:], pc_d[:])
            # iota-repeated-K operand for the mask builds, on the idle GpSimd
            # queue so it is ready before the pre DMA lands
            iotar = constp.tile([P, P * K], mybir.dt.int16)
            nc.gpsimd.iota(iotar[:], pattern=[[1, P], [0, K]], channel_multiplier=0)
            # larger consts only needed at first flush / stage 2
            ident_sb = constp.tile([P, P], bf16)
            wk_sb = constp.tile([P, 8 * D_OUT], bf16)

            masks = maskp.tile([P, NBK * P], bf16, tag="masks")
            tk0 = tkp.tile([P, NW * P], bf16, tag="tk0")
            tk1 = tkp.tile([P, NW * P], bf16, tag="tk1")

            loop_cm = (
                tc.For_i(
                    0,
                    reps,
                    1,
                    hint_engines=(
                        mybir.EngineType.PE,
                        mybir.EngineType.DVE,
                        mybir.EngineType.Activation,
                        mybir.EngineType.SP,
                    ),
                )
                if reps > 1
                else contextlib.nullcontext()
            )
            first_body = [True, True]
            with loop_cm:
                _emit_body(
                    nc, tc, mybir, AP, f32, bf16, NW, NB, NWG, blocks,
                    first_tile, last_tile, chunkp, atp, winp, tpp, pop,
                    pc_sb, iotar, ident_sb, wk_sb, masks, tk0, tk1,
                    xp_d, out_d, id_d, wk_d, first_body,
                )

    nc.compile()
    return nc


def _emit_body(
    nc, tc, mybir, AP, f32, bf16, NW, NB, NWG, blocks, first_tile, last_tile,
    chunkp, atp, winp, tpp, pop, pc_sb, iotar, ident_sb, wk_sb, masks, tk0, tk1,
    xp_d, out_d, id_d, wk_d, first_body,
):
    NBK = NB * K
    pc_t = pc_sb[:].tensor
    iot_t = iotar[:].tensor
    masks_t = masks[:].tensor

    # all mask builds up-front: depend only on pc, run while x streams.
    # out[p, c*K + k] = (pre[p, gK+k] == c)  -- k-interleaved, both TT
    # operands keep stride-1 last dims (DVE fast mode).
    for g in range(NB):
        in0 = AP(iot_t, 0, [[P * K, P], [1, P * K]])
        in1 = AP(pc_t, g * K, [[NBK, P], [0, P], [1, K]])
        nc.vector.tensor_tensor(
            out=masks[:, g * K * P : (g + 1) * K * P],
            in0=in0,
            in1=in1,
            op=mybir.AluOpType.is_equal,
        )

    quad_flushed = {g: 0 for g in range(NWG)}
    quads_done = set()
    quad_emitted = {}
    po_tiles = {}
    real_windows = set(first_tile)

    last_real_g = max(first_tile) // 4

    def emit_stage2_quad(g, w_hi=None):
        # windows [4g, 4g+4) -> 128 structs -> output rows [128g, 128g+128)
        # w_hi: restrict to windows [4g, w_hi] (partial emit for the final
        # quad so only the last window's matmuls sit in the serial tail)
        po = po_tiles.get(g)
        if po is None:
            po = po_tiles[g] = pop.tile([P, D_OUT], f32, tag="po", name=f"po{g}")
        w0 = quad_emitted.get(g, 4 * g)
        w1 = 4 * g + 4 if w_hi is None else w_hi + 1
        quad_emitted[g] = w1
        for kc, tkbuf in ((0, tk0), (1, tk1)):
            for s in range(N_SPECIES):
                nc.tensor.matmul(
                    po[32 * (w0 - 4 * g) : 32 * (w1 - 4 * g), :],
                    lhsT=tkbuf[:, w0 * P + s : w1 * P : 4],
                    rhs=wk_sb[:, (s * 2 + kc) * D_OUT : (s * 2 + kc + 1) * D_OUT],
                    start=(kc == 0 and s == 0),
                    stop=(kc == 1 and s == N_SPECIES - 1),
                    tile_position=(0, 32 * (w0 - 4 * g)) if w0 != 4 * g or w1 != 4 * g + 4 else None,
                )
        if w1 < 4 * g + 4:
            return
        quads_done.add(g)
        ob = atp.tile([P, D_OUT], bf16, tag="ob", name=f"ob{g}")
        nc.scalar.copy(ob[:], po[:])
        nc.sync.dma_start(out_d[g * P : (g + 1) * P, :], ob[:])

    # chunk plan: small ramp chunks so compute starts early, big middle
    # chunks to amortize the ~600ns DMA trigger cost, small tail chunks so
    # the final tiles (the serial tail) start as soon as possible
    plan = list(RAMP)
    body_end = TPC - TAIL_TILES
    while sum(plan) + CH <= body_end:
        plan.append(CH)
    if sum(plan) < body_end:
        plan.append(body_end - sum(plan))
    while sum(plan) + TAIL_CHUNK <= TPC:
        plan.append(TAIL_CHUNK)
    if sum(plan) < TPC:
        plan.append(TPC - sum(plan))
    starts = list(np.cumsum([0] + plan[:-1]))
    sizes = dict(zip(starts, plan))


    # block index lookup: blocks are ordered by (t, w)
    block_idx = {b: j for j, b in enumerate(blocks)}
    tile_windows = {}
    for t, w in blocks:
        tile_windows.setdefault(t, []).append(w)

    psw = {}
    chunk = None
    coff = 0
    for t in range(TPC):
        if t in sizes:
            csz = sizes[t]
            chunk = chunkp.tile([P, CH * D_IN], bf16, tag="chunk", name=f"ch{t}")
            nc.sync.dma_start(
                chunk[:, : csz * D_IN], xp_d[:, t * D_IN : (t + csz) * D_IN]
            )
            coff = t
        if t == 0 and first_body[0]:
            # ident on the scalar queue (tiny); tk zero-fill on idle GpSimd
            # so the DVE starts mask builds immediately; wk (0.5 MB) is
            # deferred to the first flush to keep it off the early stream
            first_body[0] = False
            nc.scalar.dma_start(ident_sb[:], id_d[:])
            nc.gpsimd.memset(tk0[:], 0.0)
            nc.gpsimd.memset(tk1[:], 0.0)
        xt = chunk[:, (t - coff) * D_IN : (t - coff + 1) * D_IN]
        for w in tile_windows[t]:
            if w not in psw:
                psw[w] = winp.tile([P, D_IN], f32, tag="win", name=f"win{w}")
            j = block_idx[(t, w)]
            g, k = j // K, j % K
            lhsT = AP(masks_t, g * K * P + k, [[NBK * P, P], [K, P]])
            nc.tensor.matmul(
                psw[w][:],
                lhsT=lhsT,
                rhs=xt,
                start=(t == first_tile[w]),
                stop=(t == last_tile[w]),
            )
        # flush finished windows: transpose into feature-major tk buffers
        for w in sorted(psw):
            if t != last_tile[w]:
                continue
            at = atp.tile([P, D_IN], bf16, tag="at")
            nc.scalar.copy(at[:], psw[w][:])
            if first_body[1]:
                first_body[1] = False
                nc.scalar.dma_start(wk_sb[:], wk_d[:])
            for kc, tkbuf in ((0, tk0), (1, tk1)):
                tp = tpp.tile([P, P], bf16, tag="tp")
                nc.tensor.transpose(
                    out=tp[:],
                    in_=at[:, kc * P : (kc + 1) * P],
                    identity=ident_sb[:],
                )
                nc.scalar.copy(tkbuf[:, w * P : (w + 1) * P], tp[:])
            del psw[w]
            g = w // 4
            quad_flushed[g] += 1
            n_real = sum(1 for ww in range(4 * g, 4 * g + 4) if ww in real_windows)
            if quad_flushed[g] == n_real:
                emit_stage2_quad(g)
            elif g == last_real_g and quad_flushed[g] == n_real - 1:
                # final quad: pre-emit all but its last window mid-stream
                emit_stage2_quad(g, w_hi=4 * g + n_real - 2)

    # quads with no real windows (NW padding): zeros via tk memset
    for g in range(NWG):
        if g not in quads_done:
            emit_stage2_quad(g)


def _prep(x, W, b, central_species, structural_indices):
    """Host-side prep: schedule from indices + packed per-core in_maps."""
    import ml_dtypes

    x = np.asarray(x, dtype=np.float32)
    Wf = np.asarray(W, dtype=np.float32)
    bf = np.asarray(b, dtype=np.float32)
    cs = np.asarray(central_species).astype(np.int64)
    si = np.asarray(structural_indices).astype(np.int64)

    if not np.all(np.diff(si) >= 0):
        order = np.argsort(si, kind="stable")
        si = si[order]
        cs = cs[order]
        x = x[order]

    seg = 4 * si + cs
    g0 = [int(si[c * SH]) for c in range(N_CORES)]
    seg_local_real = [
        (seg[c * SH : (c + 1) * SH] - 4 * g0[c]).astype(np.int64)
        for c in range(N_CORES)
    ]
    NW, blocks, first_tile, last_tile, max_alive = _schedule(seg_local_real)
    win_bufs = 4
    NB = (len(blocks) + K - 1) // K
    NBK = NB * K

    bf16 = ml_dtypes.bfloat16
    ident = np.eye(P, dtype=bf16)
    wk = np.zeros((P, 8, D_OUT), bf16)
    for s in range(N_SPECIES):
        for kc in range(2):
            wk[:, s * 2 + kc, :] = Wf[s, kc * P : (kc + 1) * P, :].astype(bf16)
    wk = np.ascontiguousarray(wk.reshape(P, 8 * D_OUT))

    in_maps = []
    for c in range(N_CORES):
        xp = np.zeros((SH_PAD, D_IN), np.float32)
        xp[:SH] = x[c * SH : (c + 1) * SH]
        # partition-major: SBUF partition p holds atoms {t*128+p} contiguously
        xp = np.ascontiguousarray(
            xp.reshape(TPC, P, D_IN).transpose(1, 0, 2).reshape(P, TPC * D_IN)
        ).astype(bf16)
        segt = np.full((TPC, P), -(1 << 20), np.float32)
        segt.reshape(-1)[:SH] = seg_local_real[c].astype(np.float32)
        pre = np.full((P, NBK), -1.0, np.float32)
        for j, (t, w) in enumerate(blocks):
            v = segt[t] - P * w
            pre[:, j] = np.where((v >= 0) & (v < P), v, -1.0)
        in_maps.append(
            {"xp": xp, "pc": pre.astype(np.int16), "wk": wk, "ident": ident}
        )

    # exact bias term on host: out[g] += sum_s count[g, s] * b[s]
    counts = np.bincount(seg, minlength=4 * N_STRUCT).reshape(N_STRUCT, 4)
    host_bias = counts.astype(np.float32) @ bf

    return {
        "build_args": (NW, blocks, first_tile, last_tile, win_bufs),
        "in_maps": in_maps,
        "g0": g0,
        "NW": NW,
        "host_bias": host_bias,
    }


def kernel(x, W, b, central_species, structural_indices):
    from concourse.bass_utils import run_bass_kernel_spmd

    prep = _prep(x, W, b, central_species, structural_indices)
    nc = _build(*prep["build_args"])
    res = run_bass_kernel_spmd(
        nc, prep["in_maps"], core_ids=list(range(N_CORES))
    )

    g0, NW = prep["g0"], prep["NW"]
    full = np.zeros((N_STRUCT + NW * 32, D_OUT), np.float32)
    for c in range(N_CORES):
        full[g0[c] : g0[c] + NW * 32] += np.asarray(
            res.results[c]["out"], dtype=np.float32
        )
    full[:N_STRUCT] += prep["host_bias"]
    return np.ascontiguousarray(full[:N_STRUCT])
